# revision 27
# baseline (speedup 1.0000x reference)
"""Multi-head attention (B=2, S=2048, H=2048, NH=16, HD=128) on 8 trn2 cores.

Sharding: core i -> (batch b = i // 4, head-group g = i % 4, 4 heads each).
Each core computes q/k/v projections for its 4 heads, causal-masked
attention, and a partial output projection against its 512-row slice of
Wo.  The host sums the 4 partial outputs per batch.

Layout strategy (everything K-major so no on-chip transposes are needed):
  - host ships x^T (per batch) in bf16; projections compute q^T/k^T
    [d, t] via lhsT=W, rhs=x^T, and v [T, d] via lhsT=x^T, rhs=Wv.
  - scores^T [T, t] = (k^T).T @ q^T; exp on ACT (no max subtraction --
    scores are O(6) here, exp is safe in fp32); runtime mask applied
    multiplicatively AFTER exp (so softmax denominators stay exact).
  - softmax denominators: e tiles accumulate on DVE into an fp32 esum,
    reduced across partitions with one ones-matmul per (head, block);
    o^T [d, t] = v.T @ e accumulates in PSUM; normalized by broadcast
    reciprocal on the way out to SBUF.
  - final: out[t, m] = (o^T).T @ Wo_rows, accumulated over the 4 heads.

The mask is inspected on the host and the kernel is specialized per
128x512 block: skip (all False), full (all True), or partial (loads the
mask tile and multiplies).  For the causal mask this halves attention
FLOPs; for an all-ones mask it becomes a dense kernel automatically.

Emission is software-pipelined: in query-block tau's slot we emit its
attention heads round-robin with the projections of tau+1 and deferred
output-projection rows, so the PE always has independent matmul work
while ACT grinds through the exps.
"""

import math

import numpy as np
import ml_dtypes

B, S, H, NH, HD = 2, 2048, 2048, 16, 128
N_CORES = 8
GROUPS = 4                # head-groups (cores per batch)
HPC = NH // GROUPS        # heads per core = 4
DPC = HPC * HD            # head dims per core = 512
TBLK = 512                # query-block width (matmul moving dim)
KBLK = 128                # key-block width (matmul contraction dim)
NT = S // TBLK            # 4 query blocks
NK = S // KBLK            # 16 key blocks
HKT = H // 128            # 16 contraction tiles over hidden dim
HKC = 4                   # contraction chunks per DMA (so loads pipeline)

_BF16 = ml_dtypes.bfloat16

_kernel_cache = {}


MODE_FULL, MODE_AFFINE, MODE_LOADMASK = 0, 1, 2


def _runs(blocks):
    """Group the load-mask blocks of one query block into contiguous Tb
    runs so each run loads with a single DMA."""
    runs = []
    for Tb, mode in blocks:
        if mode != MODE_LOADMASK:
            continue
        if runs and runs[-1][-1] == Tb - 1 and len(runs[-1]) < 4:
            runs[-1].append(Tb)
        else:
            runs.append([Tb])
    return runs


def _interleave(primary, fillers):
    """Round-robin: after primary unit i, its even share of fillers."""
    out = []
    n = max(len(primary), 1)
    for i, p in enumerate(primary):
        out.append(p)
        out.extend(fillers[i * len(fillers) // n:(i + 1) * len(fillers) // n])
    out.extend(fillers[len(primary) * len(fillers) // n:])
    return out


def _build(pattern):
    """Compile the SPMD program for a given mask block pattern.

    pattern: tuple over query-block tau of tuples of (Tb, partial) pairs,
    ascending in Tb, listing key blocks that have any visible entry.
    """
    import concourse.bass as bass  # noqa: F401
    import concourse.tile as tile
    from concourse import bacc, mybir

    fp32 = mybir.dt.float32
    bf16 = mybir.dt.bfloat16
    Exp = mybir.ActivationFunctionType.Exp
    inv_sqrt_hd = 1.0 / math.sqrt(HD)

    all_runs = [_runs(blocks) for blocks in pattern]
    max_run_len = max((len(r) for runs in all_runs for r in runs), default=1)
    max_runs = max((len(runs) for runs in all_runs), default=1)

    nc = bacc.Bacc("TRN2", target_bir_lowering=False, debug=False,
                   num_devices=N_CORES)
    xT = nc.dram_tensor("xT", [H, S], bf16, kind="ExternalInput")
    wq = nc.dram_tensor("wq", [H, DPC], bf16, kind="ExternalInput")
    wk = nc.dram_tensor("wk", [H, DPC], bf16, kind="ExternalInput")
    wv = nc.dram_tensor("wv", [H, DPC], bf16, kind="ExternalInput")
    wo = nc.dram_tensor("wo", [DPC, H], bf16, kind="ExternalInput")
    maskT = nc.dram_tensor("maskT", [S, S], bf16, kind="ExternalInput")
    out = nc.dram_tensor("out", [S, H], fp32, kind="ExternalOutput")
    rbc = nc.dram_tensor("rbc", [NT * HPC, TBLK], fp32)  # reciprocal bounce

    n_chunks = HKT // HKC  # 4

    with tile.TileContext(nc) as tc:
        with (
            tc.tile_pool(name="persist", bufs=1) as persist,
            tc.tile_pool(name="xt", bufs=6) as xt_pool,
            tc.tile_pool(name="masks", bufs=max(2 * max_runs, 2)) as mask_pool,
            tc.tile_pool(name="e", bufs=9) as e_pool,
            tc.tile_pool(name="outsb", bufs=4) as out_pool,
            tc.tile_pool(name="esum", bufs=7) as esum_pool,
            tc.tile_pool(name="rp", bufs=2) as r_pool,
            tc.tile_pool(name="Rp", bufs=2) as R_pool,
            tc.tile_pool(name="ps_work", bufs=3, space="PSUM") as ps_work,
            tc.tile_pool(name="ps_score", bufs=3, space="PSUM") as ps_score,
            tc.tile_pool(name="ps_acc", bufs=2, space="PSUM") as ps_acc,
            
        ):
            # --- persistent SBUF tensors -------------------------------
            # DMA queue discipline: sync carries the latency-critical
            # steady loads (xT blocks, masks), gpsimd the weights at
            # startup plus output stores, scalar only wo (emitted late --
            # it queues behind the first exps, landing well before
            # phase 3 needs it).  Never tensor: its sequencer must stay
            # dedicated to the matmul stream.
            WCH = 2  # contraction tiles per weight-load chunk
            xt0_tiles = []
            for c in range(n_chunks):
                t = xt_pool.tile([128, HKC, TBLK], bf16, tag="xt")
                nc.sync.dma_start(
                    t[:],
                    xT.ap()[c * HKC * 128:(c + 1) * HKC * 128, 0:TBLK]
                    .rearrange("(k p) t -> p k t", p=128))
                xt0_tiles.append(t)
            w_sbs = {}
            def _w_eng(name, c):
                if name == "wq":
                    return nc.gpsimd if c % 2 == 0 else nc.scalar
                return {"wk": nc.sync, "wv": nc.scalar}[name]
            for name, dram in (("wq", wq), ("wk", wk), ("wv", wv)):
                chunks = []
                for c in range(HKT // WCH):
                    t = persist.tile([128, WCH, DPC], bf16, tag=f"{name}{c}")
                    _w_eng(name, c).dma_start(
                        t[:],
                        dram.ap()[c * WCH * 128:(c + 1) * WCH * 128, :]
                        .rearrange("(k p) d -> p k d", p=128))
                    chunks.append(t)
                w_sbs[name] = chunks
            wo_sb = persist.tile([128, HPC, H], bf16, tag="wo")

            qT_sb = persist.tile([128, HPC, S], bf16, tag="qT")
            kT_sb = persist.tile([128, HPC, S], bf16, tag="kT")
            v_sb = persist.tile([128, NK, DPC], bf16, tag="v")
            oT_sb = persist.tile([128, HPC, S], bf16, tag="oT")

            ones_bf_sb = persist.tile([128, 1], bf16, tag="ones_bf")
            nc.vector.memset(ones_bf_sb[:], 1.0)
            ones_row_sb = persist.tile([1, 128], fp32, tag="ones_row")
            nc.vector.memset(ones_row_sb[:], 1.0)

            def w_chunk(name, hk):
                return w_sbs[name][hk // WCH][:, hk % WCH, :]

            xts = {0: xt0_tiles}

            def emit_xt_load(tau):
                if tau in xts:
                    return
                tsl = slice(tau * TBLK, (tau + 1) * TBLK)
                xts[tau] = []
                for c in range(n_chunks):
                    t = xt_pool.tile([128, HKC, TBLK], bf16, tag="xt")
                    nc.sync.dma_start(
                        t[:],
                        xT.ap()[c * HKC * 128:(c + 1) * HKC * 128, tsl]
                        .rearrange("(k p) t -> p k t", p=128))
                    xts[tau].append(t)

            def xt_chunk(tau, hk):
                return xts[tau][hk // HKC][:, hk % HKC, :]

            def emit_qk_proj(tau, wname, h):
                tsl = slice(tau * TBLK, (tau + 1) * TBLK)
                dst = qT_sb if wname == "wq" else kT_sb
                ps = ps_work.tile([128, TBLK], fp32, tag="ps")
                for hk in range(HKT):
                    nc.tensor.matmul(
                        ps[:],
                        lhsT=w_chunk(wname, hk)[:, h * HD:(h + 1) * HD],
                        rhs=xt_chunk(tau, hk),
                        start=(hk == 0), stop=(hk == HKT - 1))
                nc.vector.tensor_copy(out=dst[:, h, tsl], in_=ps[:])

            def emit_v_proj(tau, tb_local):
                ps = ps_work.tile([128, TBLK], fp32, tag="ps")
                for hk in range(HKT):
                    nc.tensor.matmul(
                        ps[:],
                        lhsT=xt_chunk(tau, hk)[:, tb_local * KBLK:(tb_local + 1) * KBLK],
                        rhs=w_chunk("wv", hk),
                        start=(hk == 0), stop=(hk == HKT - 1))
                nc.vector.tensor_copy(
                    out=v_sb[:, tau * (TBLK // KBLK) + tb_local, :], in_=ps[:])

            mask_tiles = {}

            def emit_mask_loads(tau):
                tsl = slice(tau * TBLK, (tau + 1) * TBLK)
                for run in all_runs[tau]:
                    mt = mask_pool.tile([128, max_run_len, TBLK], bf16,
                                        tag="mask")
                    nc.sync.dma_start(
                        mt[:, :len(run), :],
                        maskT.ap()[run[0] * KBLK:(run[-1] + 1) * KBLK, tsl]
                        .rearrange("(k p) t -> p k t", p=128))
                    for j, Tb in enumerate(run):
                        mask_tiles[(tau, Tb)] = mt[:, j, :]

            def emit_attention_head(tau, h, chunk=4):
                tsl = slice(tau * TBLK, (tau + 1) * TBLK)
                blocks = pattern[tau]
                od = ps_acc.tile([128, TBLK], fp32, tag="od")
                tree = []  # (level, tile) stack for streaming bf16 sum tree
                for i, (Tb, mode) in enumerate(blocks):
                    if i and i % chunk == 0:
                        yield
                    sp = ps_score.tile([128, TBLK], fp32, tag="sc")
                    nc.tensor.matmul(
                        sp[:],
                        lhsT=kT_sb[:, h, Tb * KBLK:(Tb + 1) * KBLK],
                        rhs=qT_sb[:, h, tsl],
                        start=True, stop=True)
                    e = e_pool.tile([128, TBLK], bf16, tag="e")
                    nc.scalar.activation(out=e[:], in_=sp[:], func=Exp,
                                         scale=inv_sqrt_hd)
                    if mode == MODE_AFFINE:
                        # zero entries where t_rel - T_rel < delta
                        # (value = -T_rel + t_rel - delta, keep if >= 0)
                        delta = Tb * KBLK - tau * TBLK
                        nc.gpsimd.affine_select(
                            out=e[:], in_=e[:],
                            compare_op=mybir.AluOpType.is_ge,
                            fill=0.0, base=-delta,
                            pattern=[[1, TBLK]], channel_multiplier=-1)
                    elif mode == MODE_LOADMASK:
                        nc.vector.tensor_mul(e[:], e[:], mask_tiles[(tau, Tb)])
                    cur, lvl = e, 0
                    while tree and tree[-1][0] == lvl:
                        _, prev = tree.pop()
                        acc = esum_pool.tile([128, TBLK], bf16, tag="esum")
                        nc.vector.tensor_add(acc[:], prev[:], cur[:])
                        cur, lvl = acc, lvl + 1
                    tree.append((lvl, cur))
                    nc.tensor.matmul(
                        od[:],
                        lhsT=v_sb[:, Tb, h * HD:(h + 1) * HD],
                        rhs=e[:],
                        start=(i == 0), stop=(i == len(blocks) - 1))
                while len(tree) > 1:
                    _, a = tree.pop()
                    _, b2 = tree.pop()
                    acc = esum_pool.tile([128, TBLK], bf16, tag="esum")
                    nc.vector.tensor_add(acc[:], a[:], b2[:])
                    tree.append((99, acc))
                esum = tree.pop()[1]
                dn = ps_score.tile([1, TBLK], fp32, tag="sc")
                nc.tensor.matmul(dn[:], lhsT=ones_bf_sb[:], rhs=esum[:],
                                 start=True, stop=True)
                r = r_pool.tile([1, TBLK], fp32, tag="r")
                nc.vector.reciprocal_approx_fast(out=r[:], in_=dn[:])
                # partition-broadcast r as a K=1 outer product on the PE:
                # R[128, t] = ones[1,128].T @ r[1, t] -- far cheaper than a
                # DMA bounce and keeps gpsimd's ucode library dedicated to
                # affine_select
                Rp = ps_score.tile([128, TBLK], fp32, tag="sc")
                nc.tensor.matmul(Rp[:], lhsT=ones_row_sb[:], rhs=r[:],
                                 start=True, stop=True)
                R = R_pool.tile([128, TBLK], fp32, tag="R")
                nc.vector.tensor_copy(out=R[:], in_=Rp[:])
                nc.vector.tensor_mul(oT_sb[:, h, tsl], od[:], R[:])

            def emit_out_half_row(tt, half):
                # half of a 128-row slab of the final projection, stored as
                # two quarter-row tiles so stores pipeline finely
                for mbl in range(H // TBLK // 2):
                    mb = half * (H // TBLK // 2) + mbl
                    ps = ps_work.tile([128, TBLK], fp32, tag="ps")
                    for h in range(HPC):
                        nc.tensor.matmul(
                            ps[:],
                            lhsT=oT_sb[:, h, tt * 128:(tt + 1) * 128],
                            rhs=wo_sb[:, h, mb * TBLK:(mb + 1) * TBLK],
                            start=(h == 0), stop=(h == HPC - 1))
                    osb = out_pool.tile([128, TBLK], fp32, tag="osb")
                    nc.vector.tensor_copy(out=osb[:], in_=ps[:])
                    nc.sync.dma_start(
                        out.ap()[tt * 128:(tt + 1) * 128,
                                 mb * TBLK:(mb + 1) * TBLK],
                        osb[:])

            def emit_out_row(tt):
                for half in range(2):
                    emit_out_half_row(tt, half)

            # ---- emission schedule -----------------------------------
            # phase3 rows of block tau are deferred: half into slot
            # tau+1, half into slot tau+2 (clamped), so the PE has filler
            # work inside the ACT-bound attention stretches, including
            # the long final one.
            rows_per_tau = TBLK // 128
            p3_assign = {t: [] for t in range(NT + 1)}  # slot -> tt list
            for ptau in range(NT):
                rows = list(range(ptau * rows_per_tau, (ptau + 1) * rows_per_tau))
                # weight the final (longest, ACT-bound) attention slot with
                # extra PE filler: everything from the two middle blocks
                # lands in the last slot.
                slot = {0: 1, 1: NT - 1, 2: NT - 1, 3: NT}.get(
                    ptau, min(ptau + 1, NT))
                if slot <= ptau:
                    slot = min(ptau + 1, NT)
                p3_assign[slot].extend(rows)

            # projections for tau=0 run standalone (startup)
            for h in range(HPC):
                emit_qk_proj(0, "wq", h)
            for h in range(HPC):
                emit_qk_proj(0, "wk", h)
            for tb in range(rows_per_tau):
                emit_v_proj(0, tb)

            emit_mask_loads(0)

            for tau in range(NT):
                fillers = []
                if tau + 1 < NT:
                    emit_xt_load(tau + 1)
                    emit_mask_loads(tau + 1)
                    fillers += [lambda h=h, t=tau + 1: emit_qk_proj(t, "wq", h)
                                for h in range(HPC)]
                    fillers += [lambda h=h, t=tau + 1: emit_qk_proj(t, "wk", h)
                                for h in range(HPC)]
                    fillers += [lambda tb=tb, t=tau + 1: emit_v_proj(t, tb)
                                for tb in range(rows_per_tau)]
                for tt in p3_assign[tau]:
                    fillers += [lambda tt=tt, hf=hf: emit_out_half_row(tt, hf)
                                for hf in range(2)]
                # pace fillers between attention chunks so the PE always
                # has independent matmuls while ACT/DVE chew the e-chain
                n_chunks_att = sum(
                    (len(pattern[tau]) + 3) // 4 for _ in range(HPC))
                fill_iter = iter(fillers)
                for h in range(HPC):
                    for _ in emit_attention_head(tau, h):
                        f = next(fill_iter, None)
                        if f is not None:
                            f()
                    f = next(fill_iter, None)
                    if f is not None:
                        f()
                for f in fill_iter:
                    f()
                if tau == 0:
                    # wo rides the scalar queue behind tau=0's exps --
                    # out of the startup bandwidth window, but done long
                    # before the first deferred phase-3 row needs it.
                    nc.scalar.dma_start(
                        wo_sb[:],
                        wo.ap().rearrange("(c p) m -> p c m", p=128))

            for tt in p3_assign[NT]:
                emit_out_row(tt)

    nc.compile()
    return nc


def _classify(mask):
    """Per 128x512 block of mask^T: skip / full / partial, unioned over
    batches.  Returns the pattern tuple, or None if some row is fully
    masked (degenerate -- reference gives uniform weights there)."""
    if not mask.any(axis=2).all():
        return None
    tr = np.arange(TBLK)[:, None]
    Tr = np.arange(KBLK)[None, :]
    pattern = []
    for tau in range(NT):
        blocks = []
        for Tb in range(NK):
            # block of mask^T[Tb*128:(Tb+1)*128, tau*512:(tau+1)*512]
            # == mask[:, tau*512:(tau+1)*512, Tb*128:(Tb+1)*128]
            blk = mask[:, tau * TBLK:(tau + 1) * TBLK,
                       Tb * KBLK:(Tb + 1) * KBLK]
            if not blk.any():
                continue
            if blk.all():
                blocks.append((Tb, MODE_FULL))
                continue
            # causal staircase? mask[t, T] = (t >= T), i.e.
            # tau*TBLK + tr >= Tb*KBLK + Tr
            stair = (tau * TBLK + tr) >= (Tb * KBLK + Tr)
            if all((blk[b] == stair).all() for b in range(blk.shape[0])):
                blocks.append((Tb, MODE_AFFINE))
            else:
                blocks.append((Tb, MODE_LOADMASK))
        pattern.append(tuple(blocks))
    return tuple(pattern)


def _reference_fallback(x, mask, Wq, Wk, Wv, Wo):
    out = np.empty((B, S, H), np.float32)
    for b in range(B):
        q = (x[b] @ Wq).reshape(S, NH, HD).transpose(1, 0, 2)
        k = (x[b] @ Wk).reshape(S, NH, HD).transpose(1, 0, 2)
        v = (x[b] @ Wv).reshape(S, NH, HD).transpose(1, 0, 2)
        s = np.einsum("htd,hTd->htT", q, k) / np.sqrt(HD)
        s = np.where(mask[b][None], s, -1e10)
        s -= s.max(-1, keepdims=True)
        w = np.exp(s)
        w /= w.sum(-1, keepdims=True)
        o = np.einsum("htT,hTd->htd", w, v)
        out[b] = o.transpose(1, 0, 2).reshape(S, NH * HD) @ Wo
    return out


def kernel(x, mask, Wq, Wk, Wv, Wo):
    x = np.asarray(x, np.float32)
    mask = np.asarray(mask).astype(bool)
    Wq = np.asarray(Wq, np.float32)
    Wk = np.asarray(Wk, np.float32)
    Wv = np.asarray(Wv, np.float32)
    Wo = np.asarray(Wo, np.float32)
    assert x.shape == (B, S, H) and mask.shape == (B, S, S)

    pattern = _classify(mask)
    if pattern is None:
        return _reference_fallback(x, mask, Wq, Wk, Wv, Wo)

    if pattern not in _kernel_cache:
        _kernel_cache[pattern] = _build(pattern)
    nc = _kernel_cache[pattern]

    xT_b = [np.ascontiguousarray(x[b].T).astype(_BF16) for b in range(B)]
    maskT_b = [np.ascontiguousarray(mask[b].T).astype(_BF16) for b in range(B)]
    wq_g = [np.ascontiguousarray(Wq[:, g * DPC:(g + 1) * DPC]).astype(_BF16)
            for g in range(GROUPS)]
    wk_g = [np.ascontiguousarray(Wk[:, g * DPC:(g + 1) * DPC]).astype(_BF16)
            for g in range(GROUPS)]
    wv_g = [np.ascontiguousarray(Wv[:, g * DPC:(g + 1) * DPC]).astype(_BF16)
            for g in range(GROUPS)]
    wo_g = [np.ascontiguousarray(Wo[g * DPC:(g + 1) * DPC, :]).astype(_BF16)
            for g in range(GROUPS)]

    in_maps = []
    for i in range(N_CORES):
        b, g = divmod(i, GROUPS)
        in_maps.append({
            "xT": xT_b[b], "maskT": maskT_b[b],
            "wq": wq_g[g], "wk": wk_g[g], "wv": wv_g[g], "wo": wo_g[g],
        })

    from concourse.bass_utils import run_bass_kernel_spmd
    res = run_bass_kernel_spmd(nc, in_maps, core_ids=list(range(N_CORES)))

    out = np.zeros((B, S, H), np.float32)
    for i in range(N_CORES):
        b = i // GROUPS
        out[b] += res.results[i]["out"]
    return out


# revision 28
# speedup vs baseline: 1.2287x; 1.2287x over previous
"""Multi-head attention (B=2, S=2048, H=2048, NH=16, HD=128) on 8 trn2 cores.

Sharding: core i -> (batch b = i // 4, head-group g = i % 4, 4 heads each).
Each core computes q/k/v projections for its 4 heads, causal-masked
attention, and a partial output projection against its 512-row slice of
Wo.  The host sums the 4 partial outputs per batch.

Layout strategy (everything K-major so no on-chip transposes are needed):
  - host ships x^T (per batch) in bf16; projections compute q^T/k^T
    [d, t] via lhsT=W, rhs=x^T, and v [T, d] via lhsT=x^T, rhs=Wv.
  - scores^T [T, t] = (k^T).T @ q^T; exp on ACT (no max subtraction --
    scores are O(6) here, exp is safe in fp32); runtime mask applied
    multiplicatively AFTER exp (so softmax denominators stay exact).
  - softmax denominators: e tiles accumulate on DVE into an fp32 esum,
    reduced across partitions with one ones-matmul per (head, block);
    o^T [d, t] = v.T @ e accumulates in PSUM; normalized by broadcast
    reciprocal on the way out to SBUF.
  - final: out[t, m] = (o^T).T @ Wo_rows, accumulated over the 4 heads.

The mask is inspected on the host and the kernel is specialized per
128x512 block: skip (all False), full (all True), or partial (loads the
mask tile and multiplies).  For the causal mask this halves attention
FLOPs; for an all-ones mask it becomes a dense kernel automatically.

Emission is software-pipelined: in query-block tau's slot we emit its
attention heads round-robin with the projections of tau+1 and deferred
output-projection rows, so the PE always has independent matmul work
while ACT grinds through the exps.
"""

import math

import numpy as np
import ml_dtypes

B, S, H, NH, HD = 2, 2048, 2048, 16, 128
N_CORES = 8
GROUPS = 4                # head-groups (cores per batch)
HPC = NH // GROUPS        # heads per core = 4
DPC = HPC * HD            # head dims per core = 512
TBLK = 512                # query-block width (matmul moving dim)
KBLK = 128                # key-block width (matmul contraction dim)
NT = S // TBLK            # 4 query blocks
NK = S // KBLK            # 16 key blocks
HKT = H // 128            # 16 contraction tiles over hidden dim
HKC = 4                   # contraction chunks per DMA (so loads pipeline)

_BF16 = ml_dtypes.bfloat16

_kernel_cache = {}


MODE_FULL, MODE_AFFINE, MODE_LOADMASK = 0, 1, 2


def _runs(blocks):
    """Group the load-mask blocks of one query block into contiguous Tb
    runs so each run loads with a single DMA."""
    runs = []
    for Tb, mode in blocks:
        if mode != MODE_LOADMASK:
            continue
        if runs and runs[-1][-1] == Tb - 1 and len(runs[-1]) < 4:
            runs[-1].append(Tb)
        else:
            runs.append([Tb])
    return runs


def _interleave(primary, fillers):
    """Round-robin: after primary unit i, its even share of fillers."""
    out = []
    n = max(len(primary), 1)
    for i, p in enumerate(primary):
        out.append(p)
        out.extend(fillers[i * len(fillers) // n:(i + 1) * len(fillers) // n])
    out.extend(fillers[len(primary) * len(fillers) // n:])
    return out


def _build(pattern):
    """Compile the SPMD program for a given mask block pattern.

    pattern: tuple over query-block tau of tuples of (Tb, partial) pairs,
    ascending in Tb, listing key blocks that have any visible entry.
    """
    import concourse.bass as bass  # noqa: F401
    import concourse.tile as tile
    from concourse import bacc, mybir

    fp32 = mybir.dt.float32
    bf16 = mybir.dt.bfloat16
    Exp = mybir.ActivationFunctionType.Exp
    inv_sqrt_hd = 1.0 / math.sqrt(HD)

    all_runs = [_runs(blocks) for blocks in pattern]
    max_run_len = max((len(r) for runs in all_runs for r in runs), default=1)
    max_runs = max((len(runs) for runs in all_runs), default=1)

    nc = bacc.Bacc("TRN2", target_bir_lowering=False, debug=False,
                   num_devices=N_CORES)
    xT = nc.dram_tensor("xT", [H, S], bf16, kind="ExternalInput")
    wq = nc.dram_tensor("wq", [H, DPC], bf16, kind="ExternalInput")
    wk = nc.dram_tensor("wk", [H, DPC], bf16, kind="ExternalInput")
    wv = nc.dram_tensor("wv", [H, DPC], bf16, kind="ExternalInput")
    wo = nc.dram_tensor("wo", [DPC, H], bf16, kind="ExternalInput")
    maskT = nc.dram_tensor("maskT", [S, S], bf16, kind="ExternalInput")
    out = nc.dram_tensor("out", [S, H], fp32, kind="ExternalOutput")
    rbc = nc.dram_tensor("rbc", [NT * HPC, TBLK], fp32)  # reciprocal bounce

    n_chunks = HKT // HKC  # 4

    with tile.TileContext(nc) as tc:
        with (
            tc.tile_pool(name="persist", bufs=1) as persist,
            tc.tile_pool(name="xt", bufs=6) as xt_pool,
            tc.tile_pool(name="masks", bufs=max(2 * max_runs, 2)) as mask_pool,
            tc.tile_pool(name="e", bufs=9) as e_pool,
            tc.tile_pool(name="outsb", bufs=4) as out_pool,
            tc.tile_pool(name="esum", bufs=7) as esum_pool,
            tc.tile_pool(name="rp", bufs=2) as r_pool,
            tc.tile_pool(name="Rp", bufs=2) as R_pool,
            tc.tile_pool(name="ps_work", bufs=3, space="PSUM") as ps_work,
            tc.tile_pool(name="ps_score", bufs=3, space="PSUM") as ps_score,
            tc.tile_pool(name="ps_acc", bufs=2, space="PSUM") as ps_acc,
            
        ):
            # --- persistent SBUF tensors -------------------------------
            # DMA queue discipline: sync carries the latency-critical
            # steady loads (xT blocks, masks), gpsimd the weights at
            # startup plus output stores, scalar only wo (emitted late --
            # it queues behind the first exps, landing well before
            # phase 3 needs it).  Never tensor: its sequencer must stay
            # dedicated to the matmul stream.
            WCH = 2  # contraction tiles per weight-load chunk
            xt0_tiles = []
            for c in range(n_chunks):
                t = xt_pool.tile([128, HKC, TBLK], bf16, tag="xt")
                nc.sync.dma_start(
                    t[:],
                    xT.ap()[c * HKC * 128:(c + 1) * HKC * 128, 0:TBLK]
                    .rearrange("(k p) t -> p k t", p=128))
                xt0_tiles.append(t)
            w_sbs = {}
            def _w_eng(name, c):
                if name == "wq":
                    return nc.gpsimd if c % 2 == 0 else nc.scalar
                return {"wk": nc.sync, "wv": nc.scalar}[name]
            for name, dram in (("wq", wq), ("wk", wk), ("wv", wv)):
                chunks = []
                for c in range(HKT // WCH):
                    t = persist.tile([128, WCH, DPC], bf16, tag=f"{name}{c}")
                    _w_eng(name, c).dma_start(
                        t[:],
                        dram.ap()[c * WCH * 128:(c + 1) * WCH * 128, :]
                        .rearrange("(k p) d -> p k d", p=128))
                    chunks.append(t)
                w_sbs[name] = chunks
            wo_sb = persist.tile([128, HPC, H], bf16, tag="wo")

            qT_sb = persist.tile([128, HPC, S], bf16, tag="qT")
            kT_sb = persist.tile([128, HPC, S], bf16, tag="kT")
            v_sb = persist.tile([128, NK, DPC], bf16, tag="v")
            oT_sb = persist.tile([128, HPC, S], bf16, tag="oT")

            ones_bf_sb = persist.tile([128, 1], bf16, tag="ones_bf")
            nc.vector.memset(ones_bf_sb[:], 1.0)

            def w_chunk(name, hk):
                return w_sbs[name][hk // WCH][:, hk % WCH, :]

            xts = {0: xt0_tiles}

            def emit_xt_load(tau):
                if tau in xts:
                    return
                tsl = slice(tau * TBLK, (tau + 1) * TBLK)
                xts[tau] = []
                for c in range(n_chunks):
                    t = xt_pool.tile([128, HKC, TBLK], bf16, tag="xt")
                    nc.sync.dma_start(
                        t[:],
                        xT.ap()[c * HKC * 128:(c + 1) * HKC * 128, tsl]
                        .rearrange("(k p) t -> p k t", p=128))
                    xts[tau].append(t)

            def xt_chunk(tau, hk):
                return xts[tau][hk // HKC][:, hk % HKC, :]

            def emit_qk_proj(tau, wname, h):
                tsl = slice(tau * TBLK, (tau + 1) * TBLK)
                dst = qT_sb if wname == "wq" else kT_sb
                ps = ps_work.tile([128, TBLK], fp32, tag="ps")
                for hk in range(HKT):
                    nc.tensor.matmul(
                        ps[:],
                        lhsT=w_chunk(wname, hk)[:, h * HD:(h + 1) * HD],
                        rhs=xt_chunk(tau, hk),
                        start=(hk == 0), stop=(hk == HKT - 1))
                nc.vector.tensor_copy(out=dst[:, h, tsl], in_=ps[:])

            def emit_v_proj(tau, tb_local):
                ps = ps_work.tile([128, TBLK], fp32, tag="ps")
                for hk in range(HKT):
                    nc.tensor.matmul(
                        ps[:],
                        lhsT=xt_chunk(tau, hk)[:, tb_local * KBLK:(tb_local + 1) * KBLK],
                        rhs=w_chunk("wv", hk),
                        start=(hk == 0), stop=(hk == HKT - 1))
                nc.vector.tensor_copy(
                    out=v_sb[:, tau * (TBLK // KBLK) + tb_local, :], in_=ps[:])

            mask_tiles = {}

            def emit_mask_loads(tau):
                tsl = slice(tau * TBLK, (tau + 1) * TBLK)
                for run in all_runs[tau]:
                    mt = mask_pool.tile([128, max_run_len, TBLK], bf16,
                                        tag="mask")
                    nc.sync.dma_start(
                        mt[:, :len(run), :],
                        maskT.ap()[run[0] * KBLK:(run[-1] + 1) * KBLK, tsl]
                        .rearrange("(k p) t -> p k t", p=128))
                    for j, Tb in enumerate(run):
                        mask_tiles[(tau, Tb)] = mt[:, j, :]

            def emit_attention_head(tau, h, chunk=4):
                tsl = slice(tau * TBLK, (tau + 1) * TBLK)
                blocks = pattern[tau]
                od = ps_acc.tile([128, TBLK], fp32, tag="od")
                tree = []  # (level, tile) stack for streaming bf16 sum tree
                for i, (Tb, mode) in enumerate(blocks):
                    if i and i % chunk == 0:
                        yield
                    sp = ps_score.tile([128, TBLK], fp32, tag="sc")
                    nc.tensor.matmul(
                        sp[:],
                        lhsT=kT_sb[:, h, Tb * KBLK:(Tb + 1) * KBLK],
                        rhs=qT_sb[:, h, tsl],
                        start=True, stop=True)
                    e = e_pool.tile([128, TBLK], bf16, tag="e")
                    nc.scalar.activation(out=e[:], in_=sp[:], func=Exp,
                                         scale=inv_sqrt_hd)
                    if mode == MODE_AFFINE:
                        # zero entries where t_rel - T_rel < delta
                        # (value = -T_rel + t_rel - delta, keep if >= 0)
                        delta = Tb * KBLK - tau * TBLK
                        nc.gpsimd.affine_select(
                            out=e[:], in_=e[:],
                            compare_op=mybir.AluOpType.is_ge,
                            fill=0.0, base=-delta,
                            pattern=[[1, TBLK]], channel_multiplier=-1)
                    elif mode == MODE_LOADMASK:
                        nc.vector.tensor_mul(e[:], e[:], mask_tiles[(tau, Tb)])
                    cur, lvl = e, 0
                    while tree and tree[-1][0] == lvl:
                        _, prev = tree.pop()
                        acc = esum_pool.tile([128, TBLK], bf16, tag="esum")
                        nc.vector.tensor_add(acc[:], prev[:], cur[:])
                        cur, lvl = acc, lvl + 1
                    tree.append((lvl, cur))
                    nc.tensor.matmul(
                        od[:],
                        lhsT=v_sb[:, Tb, h * HD:(h + 1) * HD],
                        rhs=e[:],
                        start=(i == 0), stop=(i == len(blocks) - 1))
                while len(tree) > 1:
                    _, a = tree.pop()
                    _, b2 = tree.pop()
                    acc = esum_pool.tile([128, TBLK], bf16, tag="esum")
                    nc.vector.tensor_add(acc[:], a[:], b2[:])
                    tree.append((99, acc))
                esum = tree.pop()[1]
                dn = ps_score.tile([1, TBLK], fp32, tag="sc")
                nc.tensor.matmul(dn[:], lhsT=ones_bf_sb[:], rhs=esum[:],
                                 start=True, stop=True)
                r = r_pool.tile([1, TBLK], fp32, tag="r")
                nc.vector.reciprocal_approx_fast(out=r[:], in_=dn[:])
                R = R_pool.tile([128, TBLK], fp32, tag="R")
                # partition-broadcast via a DRAM bounce (stride-0 partition
                # reads are only legal on DRAM APs) -- keeps gpsimd's ucode
                # library dedicated to affine_select (library swaps cost
                # ~6us each)
                idx = tau * HPC + h
                nc.sync.dma_start(out=rbc.ap()[idx:idx + 1, :], in_=r[:])
                bcast_src = bass.AP(
                    tensor=rbc.ap().tensor, offset=idx * TBLK,
                    ap=[[0, 128], [1, TBLK]])
                nc.sync.dma_start(out=R[:], in_=bcast_src)
                nc.vector.tensor_mul(oT_sb[:, h, tsl], od[:], R[:])

            def emit_out_half_row(tt, half):
                # half of a 128-row slab of the final projection, stored as
                # two quarter-row tiles so stores pipeline finely
                for mbl in range(H // TBLK // 2):
                    mb = half * (H // TBLK // 2) + mbl
                    ps = ps_work.tile([128, TBLK], fp32, tag="ps")
                    for h in range(HPC):
                        nc.tensor.matmul(
                            ps[:],
                            lhsT=oT_sb[:, h, tt * 128:(tt + 1) * 128],
                            rhs=wo_sb[:, h, mb * TBLK:(mb + 1) * TBLK],
                            start=(h == 0), stop=(h == HPC - 1))
                    osb = out_pool.tile([128, TBLK], fp32, tag="osb")
                    nc.vector.tensor_copy(out=osb[:], in_=ps[:])
                    nc.sync.dma_start(
                        out.ap()[tt * 128:(tt + 1) * 128,
                                 mb * TBLK:(mb + 1) * TBLK],
                        osb[:])

            def emit_out_row(tt):
                for half in range(2):
                    emit_out_half_row(tt, half)

            # ---- emission schedule -----------------------------------
            # phase3 rows of block tau are deferred: half into slot
            # tau+1, half into slot tau+2 (clamped), so the PE has filler
            # work inside the ACT-bound attention stretches, including
            # the long final one.
            rows_per_tau = TBLK // 128
            p3_assign = {t: [] for t in range(NT + 1)}  # slot -> tt list
            for ptau in range(NT):
                rows = list(range(ptau * rows_per_tau, (ptau + 1) * rows_per_tau))
                # weight the final (longest, ACT-bound) attention slot with
                # extra PE filler: everything from the two middle blocks
                # lands in the last slot.
                slot = {0: 1, 1: NT - 1, 2: NT - 1, 3: NT}.get(
                    ptau, min(ptau + 1, NT))
                if slot <= ptau:
                    slot = min(ptau + 1, NT)
                p3_assign[slot].extend(rows)

            # projections for tau=0 run standalone (startup)
            for h in range(HPC):
                emit_qk_proj(0, "wq", h)
            for h in range(HPC):
                emit_qk_proj(0, "wk", h)
            for tb in range(rows_per_tau):
                emit_v_proj(0, tb)

            emit_mask_loads(0)

            for tau in range(NT):
                fillers = []
                if tau + 1 < NT:
                    emit_xt_load(tau + 1)
                    emit_mask_loads(tau + 1)
                    fillers += [lambda h=h, t=tau + 1: emit_qk_proj(t, "wq", h)
                                for h in range(HPC)]
                    fillers += [lambda h=h, t=tau + 1: emit_qk_proj(t, "wk", h)
                                for h in range(HPC)]
                    fillers += [lambda tb=tb, t=tau + 1: emit_v_proj(t, tb)
                                for tb in range(rows_per_tau)]
                for tt in p3_assign[tau]:
                    fillers += [lambda tt=tt, hf=hf: emit_out_half_row(tt, hf)
                                for hf in range(2)]
                # pace fillers between attention chunks so the PE always
                # has independent matmuls while ACT/DVE chew the e-chain
                n_chunks_att = sum(
                    (len(pattern[tau]) + 3) // 4 for _ in range(HPC))
                fill_iter = iter(fillers)
                for h in range(HPC):
                    for _ in emit_attention_head(tau, h):
                        f = next(fill_iter, None)
                        if f is not None:
                            f()
                    f = next(fill_iter, None)
                    if f is not None:
                        f()
                for f in fill_iter:
                    f()
                if tau == 0:
                    # wo rides the scalar queue behind tau=0's exps --
                    # out of the startup bandwidth window, but done long
                    # before the first deferred phase-3 row needs it.
                    nc.scalar.dma_start(
                        wo_sb[:],
                        wo.ap().rearrange("(c p) m -> p c m", p=128))

            for tt in p3_assign[NT]:
                emit_out_row(tt)

    nc.compile()
    return nc


def _classify(mask):
    """Per 128x512 block of mask^T: skip / full / partial, unioned over
    batches.  Returns the pattern tuple, or None if some row is fully
    masked (degenerate -- reference gives uniform weights there)."""
    if not mask.any(axis=2).all():
        return None
    tr = np.arange(TBLK)[:, None]
    Tr = np.arange(KBLK)[None, :]
    pattern = []
    for tau in range(NT):
        blocks = []
        for Tb in range(NK):
            # block of mask^T[Tb*128:(Tb+1)*128, tau*512:(tau+1)*512]
            # == mask[:, tau*512:(tau+1)*512, Tb*128:(Tb+1)*128]
            blk = mask[:, tau * TBLK:(tau + 1) * TBLK,
                       Tb * KBLK:(Tb + 1) * KBLK]
            if not blk.any():
                continue
            if blk.all():
                blocks.append((Tb, MODE_FULL))
                continue
            # causal staircase? mask[t, T] = (t >= T), i.e.
            # tau*TBLK + tr >= Tb*KBLK + Tr
            stair = (tau * TBLK + tr) >= (Tb * KBLK + Tr)
            if all((blk[b] == stair).all() for b in range(blk.shape[0])):
                blocks.append((Tb, MODE_AFFINE))
            else:
                blocks.append((Tb, MODE_LOADMASK))
        pattern.append(tuple(blocks))
    return tuple(pattern)


def _reference_fallback(x, mask, Wq, Wk, Wv, Wo):
    out = np.empty((B, S, H), np.float32)
    for b in range(B):
        q = (x[b] @ Wq).reshape(S, NH, HD).transpose(1, 0, 2)
        k = (x[b] @ Wk).reshape(S, NH, HD).transpose(1, 0, 2)
        v = (x[b] @ Wv).reshape(S, NH, HD).transpose(1, 0, 2)
        s = np.einsum("htd,hTd->htT", q, k) / np.sqrt(HD)
        s = np.where(mask[b][None], s, -1e10)
        s -= s.max(-1, keepdims=True)
        w = np.exp(s)
        w /= w.sum(-1, keepdims=True)
        o = np.einsum("htT,hTd->htd", w, v)
        out[b] = o.transpose(1, 0, 2).reshape(S, NH * HD) @ Wo
    return out


def kernel(x, mask, Wq, Wk, Wv, Wo):
    x = np.asarray(x, np.float32)
    mask = np.asarray(mask).astype(bool)
    Wq = np.asarray(Wq, np.float32)
    Wk = np.asarray(Wk, np.float32)
    Wv = np.asarray(Wv, np.float32)
    Wo = np.asarray(Wo, np.float32)
    assert x.shape == (B, S, H) and mask.shape == (B, S, S)

    pattern = _classify(mask)
    if pattern is None:
        return _reference_fallback(x, mask, Wq, Wk, Wv, Wo)

    if pattern not in _kernel_cache:
        _kernel_cache[pattern] = _build(pattern)
    nc = _kernel_cache[pattern]

    xT_b = [np.ascontiguousarray(x[b].T).astype(_BF16) for b in range(B)]
    maskT_b = [np.ascontiguousarray(mask[b].T).astype(_BF16) for b in range(B)]
    wq_g = [np.ascontiguousarray(Wq[:, g * DPC:(g + 1) * DPC]).astype(_BF16)
            for g in range(GROUPS)]
    wk_g = [np.ascontiguousarray(Wk[:, g * DPC:(g + 1) * DPC]).astype(_BF16)
            for g in range(GROUPS)]
    wv_g = [np.ascontiguousarray(Wv[:, g * DPC:(g + 1) * DPC]).astype(_BF16)
            for g in range(GROUPS)]
    wo_g = [np.ascontiguousarray(Wo[g * DPC:(g + 1) * DPC, :]).astype(_BF16)
            for g in range(GROUPS)]

    in_maps = []
    for i in range(N_CORES):
        b, g = divmod(i, GROUPS)
        in_maps.append({
            "xT": xT_b[b], "maskT": maskT_b[b],
            "wq": wq_g[g], "wk": wk_g[g], "wv": wv_g[g], "wo": wo_g[g],
        })

    from concourse.bass_utils import run_bass_kernel_spmd
    res = run_bass_kernel_spmd(nc, in_maps, core_ids=list(range(N_CORES)))

    out = np.zeros((B, S, H), np.float32)
    for i in range(N_CORES):
        b = i // GROUPS
        out[b] += res.results[i]["out"]
    return out


# revision 29
# speedup vs baseline: 1.2407x; 1.0097x over previous
"""Multi-head attention (B=2, S=2048, H=2048, NH=16, HD=128) on 8 trn2 cores.

Sharding: core i -> (batch b = i // 4, head-group g = i % 4, 4 heads each).
Each core computes q/k/v projections for its 4 heads, causal-masked
attention, and a partial output projection against its 512-row slice of
Wo.  The host sums the 4 partial outputs per batch.

Layout strategy (everything K-major so no on-chip transposes are needed):
  - host ships x^T (per batch) in bf16; projections compute q^T/k^T
    [d, t] via lhsT=W, rhs=x^T, and v [T, d] via lhsT=x^T, rhs=Wv.
  - scores^T [T, t] = (k^T).T @ q^T; exp on ACT (no max subtraction --
    scores are O(6) here, exp is safe in fp32); runtime mask applied
    multiplicatively AFTER exp (so softmax denominators stay exact).
  - softmax denominators: e tiles accumulate on DVE into an fp32 esum,
    reduced across partitions with one ones-matmul per (head, block);
    o^T [d, t] = v.T @ e accumulates in PSUM; normalized by broadcast
    reciprocal on the way out to SBUF.
  - final: out[t, m] = (o^T).T @ Wo_rows, accumulated over the 4 heads.

The mask is inspected on the host and the kernel is specialized per
128x512 block: skip (all False), full (all True), or partial (loads the
mask tile and multiplies).  For the causal mask this halves attention
FLOPs; for an all-ones mask it becomes a dense kernel automatically.

Emission is software-pipelined: in query-block tau's slot we emit its
attention heads round-robin with the projections of tau+1 and deferred
output-projection rows, so the PE always has independent matmul work
while ACT grinds through the exps.
"""

import math

import numpy as np
import ml_dtypes

B, S, H, NH, HD = 2, 2048, 2048, 16, 128
N_CORES = 8
GROUPS = 4                # head-groups (cores per batch)
HPC = NH // GROUPS        # heads per core = 4
DPC = HPC * HD            # head dims per core = 512
TBLK = 512                # query-block width (matmul moving dim)
KBLK = 128                # key-block width (matmul contraction dim)
NT = S // TBLK            # 4 query blocks
NK = S // KBLK            # 16 key blocks
HKT = H // 128            # 16 contraction tiles over hidden dim
HKC = 4                   # contraction chunks per DMA (so loads pipeline)

_BF16 = ml_dtypes.bfloat16

_kernel_cache = {}


MODE_FULL, MODE_AFFINE, MODE_LOADMASK = 0, 1, 2


def _runs(blocks):
    """Group the load-mask blocks of one query block into contiguous Tb
    runs so each run loads with a single DMA."""
    runs = []
    for Tb, mode in blocks:
        if mode != MODE_LOADMASK:
            continue
        if runs and runs[-1][-1] == Tb - 1 and len(runs[-1]) < 4:
            runs[-1].append(Tb)
        else:
            runs.append([Tb])
    return runs


def _interleave(primary, fillers):
    """Round-robin: after primary unit i, its even share of fillers."""
    out = []
    n = max(len(primary), 1)
    for i, p in enumerate(primary):
        out.append(p)
        out.extend(fillers[i * len(fillers) // n:(i + 1) * len(fillers) // n])
    out.extend(fillers[len(primary) * len(fillers) // n:])
    return out


def _build(pattern):
    """Compile the SPMD program for a given mask block pattern.

    pattern: tuple over query-block tau of tuples of (Tb, partial) pairs,
    ascending in Tb, listing key blocks that have any visible entry.
    """
    import concourse.bass as bass  # noqa: F401
    import concourse.tile as tile
    from concourse import bacc, mybir

    fp32 = mybir.dt.float32
    bf16 = mybir.dt.bfloat16
    Exp = mybir.ActivationFunctionType.Exp
    inv_sqrt_hd = 1.0 / math.sqrt(HD)

    all_runs = [_runs(blocks) for blocks in pattern]
    max_run_len = max((len(r) for runs in all_runs for r in runs), default=1)
    max_runs = max((len(runs) for runs in all_runs), default=1)

    nc = bacc.Bacc("TRN2", target_bir_lowering=False, debug=False,
                   num_devices=N_CORES)
    xT = nc.dram_tensor("xT", [H, S], bf16, kind="ExternalInput")
    wq = nc.dram_tensor("wq", [H, DPC], bf16, kind="ExternalInput")
    wk = nc.dram_tensor("wk", [H, DPC], bf16, kind="ExternalInput")
    wv = nc.dram_tensor("wv", [H, DPC], bf16, kind="ExternalInput")
    wo = nc.dram_tensor("wo", [DPC, H], bf16, kind="ExternalInput")
    maskT = nc.dram_tensor("maskT", [S, S], bf16, kind="ExternalInput")
    out = nc.dram_tensor("out", [S, H], fp32, kind="ExternalOutput")
    rbc = nc.dram_tensor("rbc", [NT * HPC, TBLK], fp32)  # reciprocal bounce

    n_chunks = HKT // HKC  # 4

    with tile.TileContext(nc) as tc:
        with (
            tc.tile_pool(name="persist", bufs=1) as persist,
            tc.tile_pool(name="xt", bufs=6) as xt_pool,
            tc.tile_pool(name="masks", bufs=max(2 * max_runs, 2)) as mask_pool,
            tc.tile_pool(name="e", bufs=9) as e_pool,
            tc.tile_pool(name="outsb", bufs=4) as out_pool,
            tc.tile_pool(name="esum", bufs=7) as esum_pool,
            tc.tile_pool(name="rp", bufs=2) as r_pool,
            tc.tile_pool(name="Rp", bufs=2) as R_pool,
            tc.tile_pool(name="ps_work", bufs=3, space="PSUM") as ps_work,
            tc.tile_pool(name="ps_score", bufs=3, space="PSUM") as ps_score,
            tc.tile_pool(name="ps_acc", bufs=2, space="PSUM") as ps_acc,
            
        ):
            # --- persistent SBUF tensors -------------------------------
            # DMA queue discipline: sync carries the latency-critical
            # steady loads (xT blocks, masks), gpsimd the weights at
            # startup plus output stores, scalar only wo (emitted late --
            # it queues behind the first exps, landing well before
            # phase 3 needs it).  Never tensor: its sequencer must stay
            # dedicated to the matmul stream.
            WCH = 2  # contraction tiles per weight-load chunk
            xt0_tiles = []
            for c in range(n_chunks):
                t = xt_pool.tile([128, HKC, TBLK], bf16, tag="xt")
                nc.sync.dma_start(
                    t[:],
                    xT.ap()[c * HKC * 128:(c + 1) * HKC * 128, 0:TBLK]
                    .rearrange("(k p) t -> p k t", p=128))
                xt0_tiles.append(t)
            w_sbs = {}
            def _w_eng(name, c):
                if name == "wq":
                    return nc.gpsimd if c % 2 == 0 else nc.scalar
                return {"wk": nc.sync, "wv": nc.scalar}[name]
            for name, dram in (("wq", wq), ("wk", wk), ("wv", wv)):
                chunks = []
                for c in range(HKT // WCH):
                    t = persist.tile([128, WCH, DPC], bf16, tag=f"{name}{c}")
                    _w_eng(name, c).dma_start(
                        t[:],
                        dram.ap()[c * WCH * 128:(c + 1) * WCH * 128, :]
                        .rearrange("(k p) d -> p k d", p=128))
                    chunks.append(t)
                w_sbs[name] = chunks
            wo_sb = persist.tile([128, HPC, H], bf16, tag="wo")

            qT_sb = persist.tile([128, HPC, S], bf16, tag="qT")
            kT_sb = persist.tile([128, HPC, S], bf16, tag="kT")
            v_sb = persist.tile([128, NK, DPC], bf16, tag="v")
            oT_sb = persist.tile([128, HPC, S], bf16, tag="oT")

            ones_bf_sb = persist.tile([128, 1], bf16, tag="ones_bf")
            nc.vector.memset(ones_bf_sb[:], 1.0)

            def w_chunk(name, hk):
                return w_sbs[name][hk // WCH][:, hk % WCH, :]

            xts = {0: xt0_tiles}

            def emit_xt_load(tau):
                if tau in xts:
                    return
                tsl = slice(tau * TBLK, (tau + 1) * TBLK)
                xts[tau] = []
                for c in range(n_chunks):
                    t = xt_pool.tile([128, HKC, TBLK], bf16, tag="xt")
                    nc.sync.dma_start(
                        t[:],
                        xT.ap()[c * HKC * 128:(c + 1) * HKC * 128, tsl]
                        .rearrange("(k p) t -> p k t", p=128))
                    xts[tau].append(t)

            def xt_chunk(tau, hk):
                return xts[tau][hk // HKC][:, hk % HKC, :]

            def emit_qk_proj(tau, wname, h):
                tsl = slice(tau * TBLK, (tau + 1) * TBLK)
                dst = qT_sb if wname == "wq" else kT_sb
                ps = ps_work.tile([128, TBLK], fp32, tag="ps")
                for hk in range(HKT):
                    nc.tensor.matmul(
                        ps[:],
                        lhsT=w_chunk(wname, hk)[:, h * HD:(h + 1) * HD],
                        rhs=xt_chunk(tau, hk),
                        start=(hk == 0), stop=(hk == HKT - 1))
                nc.vector.tensor_copy(out=dst[:, h, tsl], in_=ps[:])

            def emit_v_proj(tau, tb_local):
                ps = ps_work.tile([128, TBLK], fp32, tag="ps")
                for hk in range(HKT):
                    nc.tensor.matmul(
                        ps[:],
                        lhsT=xt_chunk(tau, hk)[:, tb_local * KBLK:(tb_local + 1) * KBLK],
                        rhs=w_chunk("wv", hk),
                        start=(hk == 0), stop=(hk == HKT - 1))
                nc.vector.tensor_copy(
                    out=v_sb[:, tau * (TBLK // KBLK) + tb_local, :], in_=ps[:])

            mask_tiles = {}

            def emit_mask_loads(tau):
                tsl = slice(tau * TBLK, (tau + 1) * TBLK)
                for run in all_runs[tau]:
                    mt = mask_pool.tile([128, max_run_len, TBLK], bf16,
                                        tag="mask")
                    nc.sync.dma_start(
                        mt[:, :len(run), :],
                        maskT.ap()[run[0] * KBLK:(run[-1] + 1) * KBLK, tsl]
                        .rearrange("(k p) t -> p k t", p=128))
                    for j, Tb in enumerate(run):
                        mask_tiles[(tau, Tb)] = mt[:, j, :]

            def emit_attention_head(tau, h, chunk=3):
                tsl = slice(tau * TBLK, (tau + 1) * TBLK)
                blocks = pattern[tau]
                od = ps_acc.tile([128, TBLK], fp32, tag="od")
                tree = []  # (level, tile) stack for streaming bf16 sum tree
                for i, (Tb, mode) in enumerate(blocks):
                    if i and i % chunk == 0:
                        yield
                    sp = ps_score.tile([128, TBLK], fp32, tag="sc")
                    nc.tensor.matmul(
                        sp[:],
                        lhsT=kT_sb[:, h, Tb * KBLK:(Tb + 1) * KBLK],
                        rhs=qT_sb[:, h, tsl],
                        start=True, stop=True)
                    e = e_pool.tile([128, TBLK], bf16, tag="e")
                    nc.scalar.activation(out=e[:], in_=sp[:], func=Exp,
                                         scale=inv_sqrt_hd)
                    if mode == MODE_AFFINE:
                        # zero entries where t_rel - T_rel < delta
                        # (value = -T_rel + t_rel - delta, keep if >= 0)
                        delta = Tb * KBLK - tau * TBLK
                        nc.gpsimd.affine_select(
                            out=e[:], in_=e[:],
                            compare_op=mybir.AluOpType.is_ge,
                            fill=0.0, base=-delta,
                            pattern=[[1, TBLK]], channel_multiplier=-1)
                    elif mode == MODE_LOADMASK:
                        nc.vector.tensor_mul(e[:], e[:], mask_tiles[(tau, Tb)])
                    cur, lvl = e, 0
                    while tree and tree[-1][0] == lvl:
                        _, prev = tree.pop()
                        acc = esum_pool.tile([128, TBLK], bf16, tag="esum")
                        nc.vector.tensor_add(acc[:], prev[:], cur[:])
                        cur, lvl = acc, lvl + 1
                    tree.append((lvl, cur))
                    nc.tensor.matmul(
                        od[:],
                        lhsT=v_sb[:, Tb, h * HD:(h + 1) * HD],
                        rhs=e[:],
                        start=(i == 0), stop=(i == len(blocks) - 1))
                while len(tree) > 1:
                    _, a = tree.pop()
                    _, b2 = tree.pop()
                    acc = esum_pool.tile([128, TBLK], bf16, tag="esum")
                    nc.vector.tensor_add(acc[:], a[:], b2[:])
                    tree.append((99, acc))
                esum = tree.pop()[1]
                dn = ps_score.tile([1, TBLK], fp32, tag="sc")
                nc.tensor.matmul(dn[:], lhsT=ones_bf_sb[:], rhs=esum[:],
                                 start=True, stop=True)
                r = r_pool.tile([1, TBLK], fp32, tag="r")
                nc.vector.reciprocal_approx_fast(out=r[:], in_=dn[:])
                R = R_pool.tile([128, TBLK], fp32, tag="R")
                # partition-broadcast via a DRAM bounce (stride-0 partition
                # reads are only legal on DRAM APs) -- keeps gpsimd's ucode
                # library dedicated to affine_select (library swaps cost
                # ~6us each)
                idx = tau * HPC + h
                nc.sync.dma_start(out=rbc.ap()[idx:idx + 1, :], in_=r[:])
                bcast_src = bass.AP(
                    tensor=rbc.ap().tensor, offset=idx * TBLK,
                    ap=[[0, 128], [1, TBLK]])
                nc.sync.dma_start(out=R[:], in_=bcast_src)
                nc.vector.tensor_mul(oT_sb[:, h, tsl], od[:], R[:])

            def emit_out_half_row(tt, half):
                # half of a 128-row slab of the final projection, stored as
                # two quarter-row tiles so stores pipeline finely
                for mbl in range(H // TBLK // 2):
                    mb = half * (H // TBLK // 2) + mbl
                    ps = ps_work.tile([128, TBLK], fp32, tag="ps")
                    for h in range(HPC):
                        nc.tensor.matmul(
                            ps[:],
                            lhsT=oT_sb[:, h, tt * 128:(tt + 1) * 128],
                            rhs=wo_sb[:, h, mb * TBLK:(mb + 1) * TBLK],
                            start=(h == 0), stop=(h == HPC - 1))
                    osb = out_pool.tile([128, TBLK], fp32, tag="osb")
                    nc.vector.tensor_copy(out=osb[:], in_=ps[:])
                    nc.sync.dma_start(
                        out.ap()[tt * 128:(tt + 1) * 128,
                                 mb * TBLK:(mb + 1) * TBLK],
                        osb[:])

            def emit_out_row(tt):
                for half in range(2):
                    emit_out_half_row(tt, half)

            # ---- emission schedule -----------------------------------
            # phase3 rows of block tau are deferred: half into slot
            # tau+1, half into slot tau+2 (clamped), so the PE has filler
            # work inside the ACT-bound attention stretches, including
            # the long final one.
            rows_per_tau = TBLK // 128
            p3_assign = {t: [] for t in range(NT + 1)}  # slot -> tt list
            for ptau in range(NT):
                rows = list(range(ptau * rows_per_tau, (ptau + 1) * rows_per_tau))
                # weight the final (longest, ACT-bound) attention slot with
                # extra PE filler: everything from the two middle blocks
                # lands in the last slot.
                slot = {0: NT - 1, 1: NT - 1, 2: NT - 1, 3: NT}.get(
                    ptau, min(ptau + 1, NT))
                if slot <= ptau:
                    slot = min(ptau + 1, NT)
                p3_assign[slot].extend(rows)

            # projections for tau=0 run standalone (startup)
            for h in range(HPC):
                emit_qk_proj(0, "wq", h)
            for h in range(HPC):
                emit_qk_proj(0, "wk", h)
            for tb in range(rows_per_tau):
                emit_v_proj(0, tb)

            emit_mask_loads(0)

            for tau in range(NT):
                fillers = []
                if tau + 1 < NT:
                    emit_xt_load(tau + 1)
                    emit_mask_loads(tau + 1)
                    fillers += [lambda h=h, t=tau + 1: emit_qk_proj(t, "wq", h)
                                for h in range(HPC)]
                    fillers += [lambda h=h, t=tau + 1: emit_qk_proj(t, "wk", h)
                                for h in range(HPC)]
                    fillers += [lambda tb=tb, t=tau + 1: emit_v_proj(t, tb)
                                for tb in range(rows_per_tau)]
                for tt in p3_assign[tau]:
                    fillers += [lambda tt=tt, hf=hf: emit_out_half_row(tt, hf)
                                for hf in range(2)]
                # pace fillers between attention chunks so the PE always
                # has independent matmuls while ACT/DVE chew the e-chain
                n_chunks_att = sum(
                    (len(pattern[tau]) + 3) // 4 for _ in range(HPC))
                fill_iter = iter(fillers)
                for h in range(HPC):
                    for _ in emit_attention_head(tau, h):
                        f = next(fill_iter, None)
                        if f is not None:
                            f()
                    f = next(fill_iter, None)
                    if f is not None:
                        f()
                for f in fill_iter:
                    f()
                if tau == 0:
                    # wo rides the scalar queue behind tau=0's exps --
                    # out of the startup bandwidth window, but done long
                    # before the first deferred phase-3 row needs it.
                    nc.scalar.dma_start(
                        wo_sb[:],
                        wo.ap().rearrange("(c p) m -> p c m", p=128))

            for tt in p3_assign[NT]:
                emit_out_row(tt)

    nc.compile()
    return nc


def _classify(mask):
    """Per 128x512 block of mask^T: skip / full / partial, unioned over
    batches.  Returns the pattern tuple, or None if some row is fully
    masked (degenerate -- reference gives uniform weights there)."""
    if not mask.any(axis=2).all():
        return None
    tr = np.arange(TBLK)[:, None]
    Tr = np.arange(KBLK)[None, :]
    pattern = []
    for tau in range(NT):
        blocks = []
        for Tb in range(NK):
            # block of mask^T[Tb*128:(Tb+1)*128, tau*512:(tau+1)*512]
            # == mask[:, tau*512:(tau+1)*512, Tb*128:(Tb+1)*128]
            blk = mask[:, tau * TBLK:(tau + 1) * TBLK,
                       Tb * KBLK:(Tb + 1) * KBLK]
            if not blk.any():
                continue
            if blk.all():
                blocks.append((Tb, MODE_FULL))
                continue
            # causal staircase? mask[t, T] = (t >= T), i.e.
            # tau*TBLK + tr >= Tb*KBLK + Tr
            stair = (tau * TBLK + tr) >= (Tb * KBLK + Tr)
            if all((blk[b] == stair).all() for b in range(blk.shape[0])):
                blocks.append((Tb, MODE_AFFINE))
            else:
                blocks.append((Tb, MODE_LOADMASK))
        pattern.append(tuple(blocks))
    return tuple(pattern)


def _reference_fallback(x, mask, Wq, Wk, Wv, Wo):
    out = np.empty((B, S, H), np.float32)
    for b in range(B):
        q = (x[b] @ Wq).reshape(S, NH, HD).transpose(1, 0, 2)
        k = (x[b] @ Wk).reshape(S, NH, HD).transpose(1, 0, 2)
        v = (x[b] @ Wv).reshape(S, NH, HD).transpose(1, 0, 2)
        s = np.einsum("htd,hTd->htT", q, k) / np.sqrt(HD)
        s = np.where(mask[b][None], s, -1e10)
        s -= s.max(-1, keepdims=True)
        w = np.exp(s)
        w /= w.sum(-1, keepdims=True)
        o = np.einsum("htT,hTd->htd", w, v)
        out[b] = o.transpose(1, 0, 2).reshape(S, NH * HD) @ Wo
    return out


def kernel(x, mask, Wq, Wk, Wv, Wo):
    x = np.asarray(x, np.float32)
    mask = np.asarray(mask).astype(bool)
    Wq = np.asarray(Wq, np.float32)
    Wk = np.asarray(Wk, np.float32)
    Wv = np.asarray(Wv, np.float32)
    Wo = np.asarray(Wo, np.float32)
    assert x.shape == (B, S, H) and mask.shape == (B, S, S)

    pattern = _classify(mask)
    if pattern is None:
        return _reference_fallback(x, mask, Wq, Wk, Wv, Wo)

    if pattern not in _kernel_cache:
        _kernel_cache[pattern] = _build(pattern)
    nc = _kernel_cache[pattern]

    xT_b = [np.ascontiguousarray(x[b].T).astype(_BF16) for b in range(B)]
    maskT_b = [np.ascontiguousarray(mask[b].T).astype(_BF16) for b in range(B)]
    wq_g = [np.ascontiguousarray(Wq[:, g * DPC:(g + 1) * DPC]).astype(_BF16)
            for g in range(GROUPS)]
    wk_g = [np.ascontiguousarray(Wk[:, g * DPC:(g + 1) * DPC]).astype(_BF16)
            for g in range(GROUPS)]
    wv_g = [np.ascontiguousarray(Wv[:, g * DPC:(g + 1) * DPC]).astype(_BF16)
            for g in range(GROUPS)]
    wo_g = [np.ascontiguousarray(Wo[g * DPC:(g + 1) * DPC, :]).astype(_BF16)
            for g in range(GROUPS)]

    in_maps = []
    for i in range(N_CORES):
        b, g = divmod(i, GROUPS)
        in_maps.append({
            "xT": xT_b[b], "maskT": maskT_b[b],
            "wq": wq_g[g], "wk": wk_g[g], "wv": wv_g[g], "wo": wo_g[g],
        })

    from concourse.bass_utils import run_bass_kernel_spmd
    res = run_bass_kernel_spmd(nc, in_maps, core_ids=list(range(N_CORES)))

    out = np.zeros((B, S, H), np.float32)
    for i in range(N_CORES):
        b = i // GROUPS
        out[b] += res.results[i]["out"]
    return out
